# revision 17
# baseline (speedup 1.0000x reference)
"""Trainium2 Bass kernel for nn_KVCacheMoE (B=8, S=2048, H=1024, E=8).

Strategy: batch-parallel across the 8 NeuronCores (core c owns batch c).
The router depends only on that batch's tokens, so every core computes its
own routing weights locally and its full output shard — no collectives.

v4 design (DMA consolidation):
  - x path: 8x 1MB SWDGE cast-DMAs (fp32 DRAM -> bf16 SBUF, 2 tiles per
    group) on the otherwise-idle gpsimd queue; 8x 512KB xbar DMA-transposes
    on the scalar HWDGE ring.  xT layout [P, g, r, hj, t].
  - xsum: DVE 3D reduces over transposed groups (h already on partitions,
    so no cross-partition reduction is needed at all) + tiny folds.
  - wq0 loaded chunk-0-first ([all hj, 0:512] before [all hj, 512:1024])
    split across both HWDGE rings so expert 0 starts at ~10us; PE warmup
    matmuls cover the prefix.  Other weights staged as 1MB fp32 tiles with
    ACT casts one step behind each load.
  - Router: layer 1 = 64 bf16 LDW+MM pairs on pre-cast Wr1; expert-1 MM
    groups interleaved between router stages to keep HAM at 8/8.
  - Epilogue per [128,1024] tile on DVE: stt mt=r*psum+r*be (->bf16),
    stt acc+=relu(mt) (bf16).  acc in bf16; expert 7 writes fp32 + DMA,
    final tile split in halves to shorten the tail.
"""
import numpy as np
from contextlib import ExitStack

import concourse.bass as bass
import concourse.tile as tile
from concourse import bacc, mybir
from concourse.bass_utils import run_bass_kernel_spmd

B, S, H, E = 8, 2048, 1024, 8
N_CORES = 8
P = 128
NF = 512
F32 = mybir.dt.float32
BF16 = mybir.dt.bfloat16
AX = mybir.AxisListType
ALU = mybir.AluOpType
ACTF = mybir.ActivationFunctionType

HJ = H // P           # 8
NG = 8                # x groups (2 tiles each)
N_WARM = 26


def build_nc(s=S):
    t_tiles = s // P
    ng = s // 256

    nc = bacc.Bacc("TRN2", target_bir_lowering=False, debug=False)
    x_ap = nc.dram_tensor("x", [s, H], F32, kind="ExternalInput").ap()
    we_ap = nc.dram_tensor("We", [E, H, H], F32, kind="ExternalInput").ap()
    be_ap = nc.dram_tensor("be", [E, H], F32, kind="ExternalInput").ap()
    wr1_ap = nc.dram_tensor("Wr1", [H, H], F32, kind="ExternalInput").ap()
    br1_ap = nc.dram_tensor("br1", [H], F32, kind="ExternalInput").ap()
    wr2_ap = nc.dram_tensor("Wr2", [H, E], F32, kind="ExternalInput").ap()
    br2_ap = nc.dram_tensor("br2", [E], F32, kind="ExternalInput").ap()
    out_ap = nc.dram_tensor("out", [s, H], F32, kind="ExternalOutput").ap()

    with tile.TileContext(nc) as tc, ExitStack() as ctx:
        xsp = ctx.enter_context(tc.tile_pool(name="xsp", bufs=2))
        xbp = ctx.enter_context(tc.tile_pool(name="xbp", bufs=3))
        xtpool = ctx.enter_context(tc.tile_pool(name="xt", bufs=1))
        accpool = ctx.enter_context(tc.tile_pool(name="acc", bufs=1))
        wqpool = ctx.enter_context(tc.tile_pool(name="wq", bufs=2))
        w04p = ctx.enter_context(tc.tile_pool(name="w04", bufs=2))
        wraw = ctx.enter_context(tc.tile_pool(name="wraw", bufs=2))
        wrpool = ctx.enter_context(tc.tile_pool(name="wrp", bufs=1))
        bepool = ctx.enter_context(tc.tile_pool(name="bep", bufs=2))
        mtpool = ctx.enter_context(tc.tile_pool(name="mt", bufs=2))
        outpool = ctx.enter_context(tc.tile_pool(name="ob", bufs=2))
        rpool = ctx.enter_context(tc.tile_pool(name="rp", bufs=1))
        ps = ctx.enter_context(tc.tile_pool(name="ps", bufs=3, space="PSUM"))
        psr = ctx.enter_context(tc.tile_pool(name="psr", bufs=2, space="PSUM"))

        # ---- constants / scratch ----
        ones_row = rpool.tile([1, P], F32, tag="ones_row")
        nc.vector.memset(ones_row, 1.0)
        wsrc = rpool.tile([P, 128 + NF], BF16, tag="wsrc")
        nc.vector.memset(wsrc, 0.5)

        # persistent residents
        xT = xtpool.tile([P, ng, 2, HJ, P], BF16, tag="xT", name="xT")
        acc = [accpool.tile([P, H], BF16, tag=f"acc{i}", name=f"acc{i}")
               for i in range(t_tiles)]
        wr1b = wrpool.tile([P, HJ, H], BF16, tag="wr1b", name="wr1b")
        w2b = rpool.tile([P, E, HJ], BF16, tag="w2b")
        br1t = rpool.tile([P, HJ], F32, tag="br1t")
        br2t = rpool.tile([1, E], F32, tag="br2t")

        w2raw = rpool.tile([P, E, HJ], F32, tag="w2raw")

        def xt3(g):
            return xT[:, g].rearrange("p r j t -> p (r j) t")

        # ---- PE warmup (garbage matmuls; cover the weight-load prefix) ----
        warm_ps = psr.tile([P, NF], F32, tag="psr", name="warm")
        for _ in range(N_WARM):
            nc.tensor.matmul(warm_ps[:], wsrc[:, 0:P], wsrc[:, P:P + NF],
                             start=True, stop=True)

        # ---- expert-0 bias broadcast on gpsimd (tiny) ----
        ber0 = bepool.tile([P, H], F32, tag="ber", name="ber0")
        nc.gpsimd.dma_start(ber0[:], be_ap[0:1, :].to_broadcast([P, H]))

        xsq = {}
        xbq = {}

        def load_x(g):
            t = xsp.tile([P, 2, H], F32, tag="xs", name=f"xs{g}")
            nc.sync.dma_start(
                t[:], x_ap[bass.ts(g, 256), :].rearrange("(r p) d -> p r d", p=P))
            xsq[g] = t

        def cast_x(g):
            t = xbp.tile([P, 2, H], BF16, tag="xb", name=f"xb{g}")
            nc.scalar.copy(t[:], xsq[g][:])
            xbq[g] = t

        def transpose_x(g):
            nc.scalar.dma_start(xt3(g), xbq[g][:].rearrange("p r d -> p (r d)"),
                                transpose=True)

        # ---- expert-0 weights: x0 first, chunk-0 halves, casts on DVE+ACT
        wq0 = wqpool.tile([P, HJ, H], BF16, tag="wq", name="wq0")
        load_x(0)

        def w0load(k):
            st = w04p.tile([P, 2, H], F32, tag="w04", name=f"w0s{k}")
            eng = nc.sync if k % 2 == 0 else nc.scalar
            eng.dma_start(
                st[:],
                we_ap[0, bass.ts(k, 256), :].rearrange("(j p) d -> p j d", p=P))
            return st

        def w0cast(k, st, eng):
            dst = wq0[:, bass.ts(k, 2), :]
            if eng is nc.vector:
                eng.tensor_copy(dst, st[:])
            else:
                eng.copy(dst, st[:])

        s0 = w0load(0)          # sync ring (behind x0)
        s1 = w0load(1)          # scalar ring
        load_x(1)               # sync ring
        cast_x(0)               # ACT (before w0c1 in ACT FIFO)
        w0cast(0, s0, nc.vector)   # DVE
        w0cast(1, s1, nc.scalar)   # ACT
        transpose_x(0)          # scalar ring (between k1 and k3)
        s2 = w0load(2)          # sync ring
        s3 = w0load(3)          # scalar ring
        load_x(2)               # sync ring
        w0cast(2, s2, nc.vector)   # DVE
        w0cast(3, s3, nc.scalar)   # ACT
        cast_x(1)               # ACT
        transpose_x(1)          # scalar ring

        wq1 = wqpool.tile([P, HJ, H], BF16, tag="wq", name="wq1")
        wr1st = [None] * 4
        w1st = [None] * 4
        xs16 = []

        # ---- first x group: chunk-interleaved so tile 1 c0 runs while
        #      wq0 chunk 1 is still loading ----
        ps01 = [ps.tile([P, H], F32, tag="ps", name=f"ps01_{i}")
                for i in range(2)]
        for dc in range(2):
            for tt in range(2):
                for hj in range(HJ):
                    nc.tensor.matmul(
                        ps01[tt][:, bass.ts(dc, NF)],
                        xT[:, 0, tt, hj, :],
                        wq0[:, hj, bass.ts(dc, NF)],
                        start=(hj == 0),
                        stop=(hj == HJ - 1),
                    )
        for tt in range(2):
            mt = mtpool.tile([P, H], BF16, tag="mt")
            nc.vector.tensor_tensor(mt[:], ps01[tt][:], ber0[:], op=ALU.add)
            nc.vector.tensor_scalar_max(acc[tt][:], mt[:], 0.0)
        xr0 = rpool.tile([P, 16], F32, tag="xs16_0", name="xs16_0")
        nc.vector.reduce_sum(xr0[:], xt3(0), axis=AX.X)
        xs16.append(xr0)

        # ---- phase A: expert 0 over remaining tiles ----
        for ti in range(2, t_tiles):
            g, r = divmod(ti, 2)
            if r == 0:
                if g + 2 < ng:
                    load_x(g + 2)
                if g + 1 < ng:
                    cast_x(g + 1)
                    transpose_x(g + 1)
            if ti == 13:
                # router biases + Wr2 (small; needed only at the router)
                nc.sync.dma_start(br1t[:],
                                  br1_ap.rearrange("(p r) -> p r", r=HJ))
                nc.sync.dma_start(br2t[:],
                                  br2_ap.rearrange("(a e) -> a e", a=1))
                nc.sync.dma_start(
                    w2raw[:], wr2_ap.rearrange("(p r) e -> p r e", r=HJ))
            # Wr1 raw: 1MB per even-ti in ti=4..11 on sync ring, cast behind
            if ti in (4, 6, 8, 10):
                k = (ti - 4) // 2
                st = wraw.tile([P, 2, H], F32, tag="wr", name=f"wr1s{k}")
                nc.sync.dma_start(
                    st[:], wr1_ap[bass.ts(k, 256), :]
                    .rearrange("(j p) d -> p j d", p=P))
                wr1st[k] = st
            if ti in (6, 8, 10, 12):
                k = (ti - 6) // 2
                nc.scalar.copy(
                    wr1b[:, bass.ts(k, 2), :]
                    .rearrange("p j (r c) -> p j c r", r=HJ),
                    wr1st[k][:])
            # expert-1 raw: 1MB per even-ti in ti=8..15 on scalar ring
            if ti in (8, 10, 12, 14):
                k = (ti - 8) // 2
                st = wraw.tile([P, 2, H], F32, tag="wr", name=f"w1s{k}")
                nc.scalar.dma_start(
                    st[:], we_ap[1, bass.ts(k, 256), :]
                    .rearrange("(j p) d -> p j d", p=P))
                w1st[k] = st
            if ti in (10, 12, 14):
                k = (ti - 10) // 2
                nc.scalar.copy(wq1[:, bass.ts(k, 2), :], w1st[k][:])

            # expert-0 matmuls
            mm_ps = ps.tile([P, H], F32, tag="ps")
            for dc in range(2):
                for hj in range(HJ):
                    nc.tensor.matmul(
                        mm_ps[:, bass.ts(dc, NF)],
                        xT[:, g, r, hj, :],
                        wq0[:, hj, bass.ts(dc, NF)],
                        start=(hj == 0),
                        stop=(hj == HJ - 1),
                    )
            # unscaled epilogue: acc = relu(psum + be0); scaled by r0 at e1
            mt = mtpool.tile([P, H], BF16, tag="mt")
            nc.vector.tensor_tensor(mt[:], mm_ps[:], ber0[:], op=ALU.add)
            nc.vector.tensor_scalar_max(acc[ti][:], mt[:], 0.0)

            # xsum: one 3D reduce per group once both its tiles are done
            if r == 1:
                xr = rpool.tile([P, 16], F32, tag=f"xs16_{g}", name=f"xs16_{g}")
                nc.vector.reduce_sum(xr[:], xt3(g), axis=AX.X)
                xs16.append(xr)

        nc.scalar.copy(wq1[:, bass.ts(3, 2), :], w1st[3][:])
        nc.scalar.copy(w2b[:], w2raw[:])  # w2b[p, r, e] = Wr2[8p+r, e]

        # xsum folds -> xmean [128, 8] bf16 (h on partitions)
        xtot = rpool.tile([P, 16], F32, tag="xtot")
        nc.vector.tensor_tensor(xtot[:], xs16[0][:], xs16[1][:], op=ALU.add)
        for g in range(2, ng):
            nc.vector.tensor_tensor(xtot[:], xtot[:], xs16[g][:], op=ALU.add)
        xm8 = rpool.tile([P, HJ], F32, tag="xm8")
        nc.vector.tensor_tensor(xm8[:], xtot[:, 0:HJ], xtot[:, HJ:16], op=ALU.add)
        xmean = rpool.tile([P, HJ], BF16, tag="xmean")
        nc.vector.tensor_scalar_mul(xmean[:], xm8[:], 1.0 / s)

        # ---- router (expert-1 MM groups interleaved to keep PE busy) ----
        e1_ps = []

        def e1_group(ti):
            gq, rq = divmod(ti, 2)
            gps = ps.tile([P, H], F32, tag="ps")
            for dc in range(2):
                for hj in range(HJ):
                    nc.tensor.matmul(
                        gps[:, bass.ts(dc, NF)],
                        xT[:, gq, rq, hj, :],
                        wq1[:, hj, bass.ts(dc, NF)],
                        start=(hj == 0),
                        stop=(hj == HJ - 1),
                    )
            e1_ps.append(gps)

        e1_group(0)

        hv_ps = psr.tile([P, HJ], F32, tag="psr", name="hvps")
        for rr in range(HJ):
            for hj in range(HJ):
                nc.tensor.matmul(
                    hv_ps[:, rr:rr + 1],
                    wr1b[:, hj, bass.ts(rr, P)],
                    xmean[:, hj:hj + 1],
                    start=(hj == 0),
                    stop=(hj == HJ - 1),
                )
        hsb = rpool.tile([P, HJ], BF16, tag="hsb")
        hs1 = rpool.tile([P, HJ], F32, tag="hs1")
        nc.vector.tensor_tensor(hs1[:], hv_ps[:], br1t[:], op=ALU.add)
        nc.vector.tensor_scalar_max(hsb[:], hs1[:], 0.0)

        e1_group(1)

        lg_ps = psr.tile([1, E], F32, tag="psr", name="lgps")
        for rr in range(HJ):
            nc.tensor.matmul(lg_ps[:], hsb[:, rr:rr + 1], w2b[:, rr, :],
                             start=(rr == 0), stop=(rr == HJ - 1))
        logits = rpool.tile([1, E], F32, tag="logits")
        nc.vector.tensor_tensor(logits[:], lg_ps[:], br2t[:], op=ALU.add)
        mx = rpool.tile([1, 1], F32, tag="mx")
        nc.vector.reduce_max(mx[:], logits[:], axis=AX.X)
        nmx = rpool.tile([1, 1], F32, tag="nmx")
        nc.vector.tensor_scalar_mul(nmx[:], mx[:], -1.0)
        ex = rpool.tile([1, E], F32, tag="ex")
        nc.scalar.activation(ex[:], logits[:], ACTF.Exp, bias=nmx[:], scale=1.0)
        sm = rpool.tile([1, 1], F32, tag="sm")
        nc.vector.reduce_sum(sm[:], ex[:], axis=AX.X)
        rinv = rpool.tile([1, 1], F32, tag="rinv")
        nc.vector.reciprocal(rinv[:], sm[:])
        rvec = rpool.tile([1, E], F32, tag="rvec")
        nc.vector.tensor_scalar_mul(rvec[:], ex[:], rinv[:])

        e1_group(2)

        rsb_ps = psr.tile([P, E], F32, tag="psr", name="rsbps")
        nc.tensor.matmul(rsb_ps[:], ones_row[:], rvec[:], start=True, stop=True)
        rsb = rpool.tile([P, E], F32, tag="rsb")
        nc.scalar.copy(rsb[:], rsb_ps[:])

        # ---- experts 1..7 ----
        wq = wq1
        for e in range(1, E):
            ber = bepool.tile([P, H], F32, tag="ber")
            nc.gpsimd.dma_start(ber[:], be_ap[e:e + 1, :].to_broadcast([P, H]))
            bep = bepool.tile([P, H], F32, tag="bepf")
            nc.scalar.mul(bep[:], ber[:], rsb[:, e:e + 1])

            if e < E - 1:
                wq_next = wqpool.tile([P, HJ, H], BF16, tag="wq",
                                      name=f"wq{e + 1}")
            wnr = [None] * 4
            for ti in range(t_tiles):
                g, r = divmod(ti, 2)
                if e == 1:
                    # deferred expert-0 routing weight (ACT, off the DVE path)
                    nc.scalar.mul(acc[ti][:], acc[ti][:], rsb[:, 0:1])
                if e == 1 and ti < 3:
                    mm_ps = e1_ps[ti]
                else:
                    mm_ps = ps.tile([P, H], F32, tag="ps")
                    for dc in range(2):
                        for hj in range(HJ):
                            nc.tensor.matmul(
                                mm_ps[:, bass.ts(dc, NF)],
                                xT[:, g, r, hj, :],
                                wq[:, hj, bass.ts(dc, NF)],
                                start=(hj == 0),
                                stop=(hj == HJ - 1),
                            )
                if e < E - 1 and ti in (0, 2, 4, 6):
                    k = ti // 2
                    st = wraw.tile([P, 2, H], F32, tag="wr",
                                   name=f"w{e + 1}s{k}")
                    nc.scalar.dma_start(
                        st[:], we_ap[e + 1, bass.ts(k, 256), :]
                        .rearrange("(j p) d -> p j d", p=P))
                    wnr[k] = st
                if e < E - 1 and ti in (2, 4, 6, 8):
                    k = (ti - 2) // 2
                    nc.scalar.copy(wq_next[:, bass.ts(k, 2), :], wnr[k][:])

                last_tile = (e == E - 1 and ti == t_tiles - 1)
                if not last_tile:
                    mt = mtpool.tile([P, H], BF16, tag="mt")
                    nc.vector.scalar_tensor_tensor(
                        mt[:], mm_ps[:], rsb[:, e:e + 1], bep[:],
                        op0=ALU.mult, op1=ALU.add,
                    )
                    if e < E - 1:
                        nc.vector.scalar_tensor_tensor(
                            acc[ti][:], mt[:], 0.0, acc[ti][:],
                            op0=ALU.max, op1=ALU.add,
                        )
                    else:
                        obuf = outpool.tile([P, H], F32, tag="ob")
                        nc.vector.scalar_tensor_tensor(
                            obuf[:], mt[:], 0.0, acc[ti][:],
                            op0=ALU.max, op1=ALU.add,
                        )
                        nc.sync.dma_start(out_ap[bass.ts(ti, P), :], obuf[:])
                else:
                    # split the final tile in halves to shorten the tail
                    obuf = outpool.tile([P, H], F32, tag="ob")
                    for half in range(2):
                        hsl = bass.ts(half, NF)
                        mth = mtpool.tile([P, NF], BF16, tag="mth")
                        nc.vector.scalar_tensor_tensor(
                            mth[:], mm_ps[:, hsl], rsb[:, e:e + 1], bep[:, hsl],
                            op0=ALU.mult, op1=ALU.add,
                        )
                        nc.vector.scalar_tensor_tensor(
                            obuf[:, hsl], mth[:], 0.0, acc[ti][:, hsl],
                            op0=ALU.max, op1=ALU.add,
                        )
                        nc.sync.dma_start(out_ap[bass.ts(ti, P), hsl],
                                          obuf[:, hsl])
            if e < E - 1:
                wq = wq_next

    nc.compile()
    return nc


_nc_cache = {}


def _get_nc(s):
    if s not in _nc_cache:
        _nc_cache[s] = build_nc(s)
    return _nc_cache[s]


def kernel(x, We, be, Wr1, br1, Wr2, br2):
    x = np.ascontiguousarray(np.asarray(x, dtype=np.float32))
    We = np.ascontiguousarray(np.asarray(We, dtype=np.float32))
    be = np.ascontiguousarray(np.asarray(be, dtype=np.float32))
    Wr1 = np.ascontiguousarray(np.asarray(Wr1, dtype=np.float32))
    br1 = np.ascontiguousarray(np.asarray(br1, dtype=np.float32))
    Wr2 = np.ascontiguousarray(np.asarray(Wr2, dtype=np.float32))
    br2 = np.ascontiguousarray(np.asarray(br2, dtype=np.float32))

    s = x.shape[1]
    nc = _get_nc(s)
    shared = {"We": We, "be": be, "Wr1": Wr1, "br1": br1, "Wr2": Wr2, "br2": br2}
    in_maps = [{"x": x[c], **shared} for c in range(N_CORES)]
    res = run_bass_kernel_spmd(nc, in_maps, list(range(N_CORES)))
    return np.stack([res.results[c]["out"] for c in range(N_CORES)], axis=0)
